# revision 51
# baseline (speedup 1.0000x reference)
"""Multi-head attention (B=2, S=2048, D=1024, H=16) on 8 Trainium2 cores.

Sharding: 2-way data parallel over batch x 4-way tensor parallel over heads.
Each core owns one batch and 4 heads (a 256-feature slice).  Per core:
  - QKV projections for its feature slice over its batch's 2048 tokens
  - causal attention for its 4 heads with block-skipping
  - partial output projection (contraction over its 256 features)
Host: transposes/splits inputs, sums the 4 partial outputs per batch, adds
bo (+ Wo@bv: softmax rows sum to 1, so the v bias is a constant shift).

On-chip pipeline:
  - QKV projections run as fp8e4 DoubleRow matmuls (0.5 cycles/row) with a
    hi+lo 3-term split (x8*w8 + x8*wr + xr*w8) prepared on the host;
    weights pre-scaled x32 to keep residuals out of the fp8 subnormal
    range.  0.75x the bf16 PE time at ~2x BETTER accuracy.
  - v is projected DIRECTLY into token-major layout (lhsT = the v input
    tile, moving dim = features) -- no PE transposes.
  - scores run float32r at full PE rate on fp32r qh/kh (moving >= 256);
    probs/v are bf16 (fp8 would blow the 2e-2 error budget).
  - causal masking: GPSIMD affine_select on just the 128-column window
    straddling the diagonal; fully-masked tiles skipped, trims applied.
  - softmax row-sums come free from ones-columns in the PV lhsT; 1/l is
    broadcast across partitions by K=1 PE matmuls against a one-row
    selector (GPSIMD cannot read psum; partition_broadcast is broken).
  - masked-tile PVs release 2 tiles late, plain-tile PVs 1 tile late, so
    nothing not-yet-ready ever blocks ready score matmuls in the in-order
    PE stream.
Cross-phase work (next-block projections + DMAs, finished-block output
projections) is interleaved into the attention stream with strict budget
pacing; the last n-block's output projection is split by feature-half so
one half runs during the final attention block, and reserved outproj
units are emitted just before the final normalize to keep PE hot through
the drain.  Startup: the k-path DMAs go first (wk quarter 0, k quarter 0,
rest of wk merged), wq/wv load as single DMAs, and the 1MB wo transfer is
deferred into the filler stream (slot 40, among block-3's input DMAs) so
block-0/1/2 input quarters are never queued behind it on the DMA engine.
PV emission trails the score/exp stream by PV_LAG=4 tiles (deferred
masked tiles by DEF_LAG=4): the ACT engine -- which saturates in the
causally-backloaded second half -- gets a 4-tile exp backlog cushion, so
PV matmuls almost never stall the in-order PE stream on a pending exp.
The final block runs lag 2/2 instead so its PV burst doesn't push the
drain's normalize chain later.  (A score-hoisting mechanism exists behind
MHA_HOIST -- emitting future blocks' score+exp as low-priority filler --
but every setting regressed: a hoisted score pins one of the two psum
score-ring slots until its exp drains through the ACT backlog, which in
the ACT-saturated second half stalls the in-order PE stream more than the
overlap gains.  Default 0.)
The softmax 1/l broadcast is a single K=128 selector matmul per block
(dead selector/staging rows zeroed once at startup; the staging tile is
persistent so the zeros survive all blocks).
Validated: 129657 ns (TimelineSim), rel err 4.6e-3 on HW.
"""

import os

import numpy as np
import ml_dtypes

D_MODEL = 1024
NUM_HEADS = 16
DEPTH = 64
BATCH = 2
SEQ = 2048
N_CORES = 8
H_LOC = 4  # heads per core
FW = 256  # features per core (4 heads x 64)
P = 128
SB = 512  # s-block width
NTOK_LOC = SEQ  # tokens per core (one batch)
N_SB = SEQ // SB  # 4 s-blocks
N_TT = SEQ // P  # 16 t-tiles
N_NB = NTOK_LOC // SB  # 4 n-blocks
N_CT = D_MODEL // P  # 8 contraction tiles
N_FT = FW // P  # 2 feature tiles
N_HP = H_LOC // 2  # 2 head-pairs

NORM_MODE = os.environ.get("MHA_NORM", "mmb")  # mmb | dma
# fp8 3-term projections: inputs/weights are split hi+lo into fp8e4 on the
# host; each 128-deep contraction tile needs 3 of the 4 cross products
# (x8*w8, x8*wr, xr*w8 -- the dropped xr*wr term is O(quant_err^2)), and
# DoubleRow packs 2 products per matmul at 0.5 cycles/row: 12 instructions
# replace 8 at 0.75x the PE time with ~2x BETTER accuracy than bf16 inputs.
# Weights are pre-scaled by W_SCALE so their residuals stay out of the
# fp8 subnormal range; the scale is folded into the exp scale and Wo.
FP8 = os.environ.get("MHA_FP8", "1") == "1"
W_SCALE = 32.0
# filler pacing costs (in score-tile-iteration credits)
C_DMA = int(os.environ.get("MHA_C_DMA", "1"))
C_QK = int(os.environ.get("MHA_C_QK", "2"))
C_V = int(os.environ.get("MHA_C_V", "2"))
C_OP = int(os.environ.get("MHA_C_OP", "3"))
PV_LAG = int(os.environ.get("MHA_PV_LAG", "4"))
DEF_LAG = int(os.environ.get("MHA_DEF_LAG", "4"))

LAST_RESULTS = None
LAST_EXEC_WALL = None


def _mask_structure(mask_np):
    """Classify each (t-tile, s-block) of the [S, S] mask (1.0 = disallowed).

    Returns (kind, mix_idx, patterns): kind[i][j] in
    {"skip", "plain", "affine", "mixed"}; for "affine", mix_idx[i][j] is the
    offset c of keep = (s >= c + t); for "mixed" it indexes into patterns
    (list of [P, SB] keep-masks).  mask rows = query s, cols = key t;
    scoresT is [t, s] so we transpose.
    """
    maskT = np.ascontiguousarray(mask_np.reshape(SEQ, SEQ).T)
    kind = [[None] * N_SB for _ in range(N_TT)]
    mix_idx = [[None] * N_SB for _ in range(N_TT)]
    patterns = []
    pat_key = {}
    s_idx = np.arange(SB)[None, :]
    t_idx = np.arange(P)[:, None]
    for i in range(N_TT):
        for j in range(N_SB):
            sub = maskT[i * P : (i + 1) * P, j * SB : (j + 1) * SB]
            if np.all(sub >= 0.5):
                kind[i][j] = "skip"
                continue
            if np.all(sub < 0.5):
                kind[i][j] = "plain"
                continue
            keep = (sub < 0.5).astype(np.float32)
            first_one = np.argmax(keep, axis=1)
            c = int(first_one[0])
            if np.array_equal(keep, (s_idx >= c + t_idx).astype(np.float32)):
                kind[i][j] = "affine"
                mix_idx[i][j] = c
                continue
            kind[i][j] = "mixed"
            key = keep.tobytes()
            if key not in pat_key:
                pat_key[key] = len(patterns)
                patterns.append(keep)
            mix_idx[i][j] = pat_key[key]
    return kind, mix_idx, patterns


def _build_nc(kind, mix_idx, n_patterns, has_bias):
    import concourse.tile as tile
    import concourse.mybir as mybir
    from concourse import bacc

    dt = mybir.dt

    nc = bacc.Bacc(None, target_bir_lowering=False)

    if FP8:
        # inputs ship as (hi, lo) fp8e4 pairs interleaved per 512-token
        # block (so DMA slices stay <= 3 dims); weights interleave
        # (lo, hi) at full width.  The orders make the DoubleRow slot APs
        # of all three product groups regular-stride slices.
        IN_DT = dt.float8e4
        in_shape = [D_MODEL, 2 * NTOK_LOC]
        w_shape = [D_MODEL, 2 * FW]
    else:
        IN_DT = dt.bfloat16
        in_shape = [D_MODEL, NTOK_LOC]
        w_shape = [D_MODEL, FW]
    qT = nc.dram_tensor("qT", in_shape, IN_DT, kind="ExternalInput")
    kT = nc.dram_tensor("kT", in_shape, IN_DT, kind="ExternalInput")
    vT = nc.dram_tensor("vT", in_shape, IN_DT, kind="ExternalInput")
    wq = nc.dram_tensor("wqT", w_shape, IN_DT, kind="ExternalInput")
    wk = nc.dram_tensor("wkT", w_shape, IN_DT, kind="ExternalInput")
    wv = nc.dram_tensor("wvT", w_shape, IN_DT, kind="ExternalInput")
    wo = nc.dram_tensor("woT", [FW, D_MODEL], dt.float32r, kind="ExternalInput")
    maskt = None
    if n_patterns:
        maskt = nc.dram_tensor(
            "maskt", [n_patterns, P, SB], dt.bfloat16, kind="ExternalInput"
        )
    bq = bk = bv = None
    if has_bias:
        bq = nc.dram_tensor("bq", [FW, 1], dt.float32, kind="ExternalInput")
        bk = nc.dram_tensor("bk", [FW, 1], dt.float32, kind="ExternalInput")
        bv = nc.dram_tensor("bv", [FW, 1], dt.float32, kind="ExternalInput")
    # cols [NTOK_LOC, NTOK_LOC+SB) hold the kt=1 partial of the last
    # n-block (its outproj is split by feature-half so the first half can
    # run during the final attention block); host sums the two partials.
    outT = nc.dram_tensor(
        "outT", [D_MODEL, NTOK_LOC + SB], dt.bfloat16, kind="ExternalOutput"
    )
    outT_r = outT.rearrange("(dt p) n -> p dt n", p=P)

    with tile.TileContext(nc) as tc:
        with (
            tc.tile_pool(name="const", bufs=1) as const,
            tc.tile_pool(name="big", bufs=1) as big,
            tc.tile_pool(name="stageA", bufs=2) as stA,
            tc.tile_pool(name="expp", bufs=6) as expp,
            tc.tile_pool(name="linp", bufs=2) as linp,
            # PSUM: mm 2x1 banks + sc 2x2 banks + pv 2x1 = 8 banks
            tc.tile_pool(name="mmps", bufs=2, space="PSUM") as mmps,
            tc.tile_pool(name="scps", bufs=2, space="PSUM") as scps,
            tc.tile_pool(name="pvps", bufs=1, space="PSUM") as pvps,
        ):
            # ---- constants (tiles only; DMAs are interleaved with the
            # first token-block loads in the driver so the first k-proj
            # matmul is not queued behind 6us of weight transfers) ----
            w_sb_shape = [P, N_CT, 2, FW] if FP8 else [P, N_CT, FW]
            wq_sb = const.tile(w_sb_shape, IN_DT)
            wk_sb = const.tile(w_sb_shape, IN_DT)
            wv_sb = const.tile(w_sb_shape, IN_DT)
            wo_sb = const.tile([P, N_FT, N_CT, P], dt.float32r)
            mask_sb = None
            if n_patterns:
                mask_sb = const.tile([P, n_patterns, SB], dt.bfloat16)
            bias_sb = {}
            if has_bias:
                for name in ("q", "k"):
                    bias_sb[name] = const.tile(
                        [P, N_FT], dt.float32, name=f"b{name}"
                    )

            # ---- persistent activations ----
            # bf16 qh/kh run the score matmuls at the same PE rate but
            # without fp32r's >=256-moving-dim constraint, so diagonal
            # tiles trim fully; the ~0.2% quantization is far under budget
            qh_sb = big.tile([P, N_FT, NTOK_LOC], dt.bfloat16)
            kh_sb = big.tile([P, N_FT, NTOK_LOC], dt.bfloat16)
            ao_sb = big.tile([P, N_FT, NTOK_LOC], dt.float32r)
            # selector for the l-broadcast matmul: sel[64, 0:64]=1 routes
            # ltmp row 64 (1/l_a) -> psum rows 0:64; sel[0, 64:128]=1
            # routes row 0 (1/l_b) -> rows 64:128.  ONE K=65 matmul (cost
            # is moving-dim only: 213ns) replaces two K=1 matmuls; all
            # dead rows of sel AND of both ltmp ring buffers are zeroed
            # once at startup so the full-128 contraction (partial ranges
            # fail the BIR verifier) contributes exact zeros elsewhere.
            sel = const.tile([P, P], dt.float32r, name="sel")
            if NORM_MODE == "mmb":
                # memset can't emit fp32r: memset an fp32 scratch, then a
                # DVE copy (a legal fp32r producer) rounds it over
                selt = const.tile([P, P], dt.float32, name="selt")
                nc.vector.memset(selt, 0.0)
                nc.vector.memset(selt[64:65, 0:64], 1.0)
                nc.vector.memset(selt[0:1, 64:128], 1.0)
                nc.vector.tensor_copy(sel, selt)
                # persistent 1/l staging row-pair: one allocation, dead
                # rows zeroed once; each block's recips overwrite only
                # rows 64 (1/l_a) and 0 (1/l_b)
                ltmp_p = big.tile([P, SB], dt.float32r, name="ltmp_p")
                zscr = const.tile([P, SB], dt.float32, name="zscr")
                nc.vector.memset(zscr, 0.0)
                # partition slices must start at 0/32/64: zero the whole
                # tile; the per-block recips then own rows 0 and 64
                nc.vector.tensor_copy(ltmp_p, zscr)
            # per head-pair: [t', t-tile, group, 64] bf16 with groups
            # [vh_a | ones | vh_b]:
            #   h0: lhsT = groups 0:2 = [v_a | 1] -> psum [data(0:64); l(64:128)]
            #   h1: lhsT = groups 1:3 = [1 | v_b] -> psum [l(0:64); data(64:128)]
            # (the 4D shape lets one strided copy fill both data groups)
            vh_sb = [
                big.tile([P, N_TT, 3, 64], dt.bfloat16, name=f"vh{hp}")
                for hp in range(N_HP)
            ]
            for hp in range(N_HP):
                nc.vector.memset(vh_sb[hp][:, :, 1, :], 1.0)

            def copyback(dst_ap, ps, bias_ap):
                if bias_ap is not None:
                    nc.vector.tensor_tensor(
                        dst_ap, ps, bias_ap.to_broadcast(ps.shape),
                        mybir.AluOpType.add,
                    )
                else:
                    nc.vector.tensor_copy(dst_ap, ps)

            def load_st(src, nb, tag):
                """DMA one 512-token block of one input, in four quarters
                (the ct-loop consumer starts after the first lands)."""
                nsl = slice(nb * SB, (nb + 1) * SB)
                quarters = []
                for h in range(4):
                    if FP8:
                        sth = stA.tile(
                            [P, 2, 2, SB], IN_DT, tag=f"{tag}{h}", name=f"{tag}{h}"
                        )
                        nc.sync.dma_start(
                            sth,
                            src.rearrange("(ct p) n2 -> p ct n2", p=P)[
                                :,
                                2 * h : 2 * h + 2,
                                nb * 2 * SB : (nb + 1) * 2 * SB,
                            ],
                        )
                    else:
                        sth = stA.tile(
                            [P, 2, SB], IN_DT, tag=f"{tag}{h}", name=f"{tag}{h}"
                        )
                        nc.sync.dma_start(
                            sth,
                            src[:, nsl].rearrange("(ct p) n -> p ct n", p=P)[
                                :, 2 * h : 2 * h + 2, :
                            ],
                        )
                    quarters.append(sth)
                return quarters

            DR = mybir.MatmulPerfMode.DoubleRow

            def project_qk_g1(halves, w_sb, ft):
                fsl = slice(ft * P, (ft + 1) * P)
                ps = mmps.tile([P, SB], dt.float32, tag="ps", name="ps")
                for h in range(4):
                    nc.tensor.matmul(
                        ps,
                        lhsT=w_sb[:, 2 * h : 2 * h + 2, 1, fsl],
                        rhs=halves[h][:, :, 0, :],
                        start=(h == 0),
                        stop=False,
                        perf_mode=DR,
                    )
                return ps

            def project_qk_g2(halves, w_sb, dst, bn, nb, ft, ps):
                fsl = slice(ft * P, (ft + 1) * P)
                for ct in range(N_CT):
                    nc.tensor.matmul(
                        ps,
                        lhsT=w_sb[:, ct, :, fsl],
                        rhs=halves[ct // 2][:, ct % 2, :, :],
                        start=False,
                        stop=(ct == N_CT - 1),
                        perf_mode=DR,
                    )
                bias_ap = (
                    bias_sb[bn][:, ft : ft + 1] if (has_bias and bn) else None
                )
                copyback(dst[:, ft, nb * SB : (nb + 1) * SB], ps, bias_ap)

            def project_qk(halves, w_sb, dst, bn, nb, ft):
                fsl = slice(ft * P, (ft + 1) * P)
                ps = mmps.tile([P, SB], dt.float32, tag="ps", name="ps")
                if FP8:
                    # G1: x8*w8 paired over each quarter's 2 k-tiles
                    for h in range(4):
                        nc.tensor.matmul(
                            ps,
                            lhsT=w_sb[:, 2 * h : 2 * h + 2, 1, fsl],
                            rhs=halves[h][:, :, 0, :],
                            start=(h == 0),
                            stop=False,
                            perf_mode=DR,
                        )
                    # G2: (wr, w8) x (x8, xr) per k-tile = x8*wr + xr*w8
                    for ct in range(N_CT):
                        nc.tensor.matmul(
                            ps,
                            lhsT=w_sb[:, ct, :, fsl],
                            rhs=halves[ct // 2][:, ct % 2, :, :],
                            start=False,
                            stop=(ct == N_CT - 1),
                            perf_mode=DR,
                        )
                else:
                    for ct in range(N_CT):
                        nc.tensor.matmul(
                            ps,
                            lhsT=w_sb[:, ct, fsl],
                            rhs=halves[ct // 2][:, ct % 2, :],
                            start=(ct == 0),
                            stop=(ct == N_CT - 1),
                        )
                bias_ap = (
                    bias_sb[bn][:, ft : ft + 1] if (has_bias and bn) else None
                )
                copyback(dst[:, ft, nb * SB : (nb + 1) * SB], ps, bias_ap)

            def project_v_g1(halves, m):
                msl = slice(m * P, (m + 1) * P)
                ps = mmps.tile([P, N_HP, 2, 64], dt.float32, tag="ps", name="psv")
                for h in range(4):
                    nc.tensor.matmul(
                        ps,
                        lhsT=halves[h][:, :, 0, msl],
                        rhs=wv_sb[:, 2 * h : 2 * h + 2, 1, :],
                        start=(h == 0),
                        stop=False,
                        perf_mode=DR,
                    )
                return ps

            def project_v_g2(halves, nb, m, ps):
                tt = nb * 4 + m
                msl = slice(m * P, (m + 1) * P)
                for ct in range(N_CT):
                    nc.tensor.matmul(
                        ps,
                        lhsT=halves[ct // 2][:, ct % 2, :, msl],
                        rhs=wv_sb[:, ct, :, :],
                        start=False,
                        stop=(ct == N_CT - 1),
                        perf_mode=DR,
                    )
                for hp in range(N_HP):
                    nc.vector.tensor_copy(
                        vh_sb[hp][:, tt, 0:3:2, :], ps[:, hp, :, :]
                    )

            def project_v(halves, nb, m):
                """Project tokens [nb*SB + m*P, +P) of v directly into the
                token-major vh tiles (lhsT = the v input tile; moving dim =
                the 256 output features)."""
                tt = nb * 4 + m
                msl = slice(m * P, (m + 1) * P)
                ps = mmps.tile([P, N_HP, 2, 64], dt.float32, tag="ps", name="psv")
                if FP8:
                    for h in range(4):
                        nc.tensor.matmul(
                            ps,
                            lhsT=halves[h][:, :, 0, msl],
                            rhs=wv_sb[:, 2 * h : 2 * h + 2, 1, :],
                            start=(h == 0),
                            stop=False,
                            perf_mode=DR,
                        )
                    for ct in range(N_CT):
                        nc.tensor.matmul(
                            ps,
                            lhsT=halves[ct // 2][:, ct % 2, :, msl],
                            rhs=wv_sb[:, ct, :, :],
                            start=False,
                            stop=(ct == N_CT - 1),
                            perf_mode=DR,
                        )
                else:
                    for ct in range(N_CT):
                        nc.tensor.matmul(
                            ps,
                            lhsT=halves[ct // 2][:, ct % 2, msl],
                            rhs=wv_sb[:, ct, :],
                            start=(ct == 0),
                            stop=(ct == N_CT - 1),
                        )
                # v bias is folded on the host: softmax rows sum to 1, so
                # bv contributes the constant bv @ Wo.T to the output.
                # One strided copy per head-pair fills both data groups
                # (dest groups {0, 2} via a step-2 slice).
                for hp in range(N_HP):
                    nc.vector.tensor_copy(
                        vh_sb[hp][:, tt, 0:3:2, :], ps[:, hp, :, :]
                    )

            def flush_filler_upto(key):
                while filler and (filler[0][2] is None or filler[0][2] <= key):
                    unit, cost, _ = filler.popleft()
                    unit()

            def attention_block(hp, j):
                flush_filler_upto((j, hp))
                ilist = [i for i in range(N_TT) if kind[i][j] != "skip"]
                assert ilist, "fully-masked s-block unsupported"
                pv = [
                    pvps.tile([P, SB], dt.float32, tag=f"pv{h}", name=f"pv{h}")
                    for h in range(2)
                ]
                ssl = slice(j * SB, (j + 1) * SB)
                n_pv = len(ilist)
                pv_emitted = 0
                deferred = []

                def emit_pv(i, e, s0):
                    nonlocal pv_emitted
                    for h in range(2):
                        nc.tensor.matmul(
                            pv[h][:, s0:],
                            lhsT=vh_sb[hp][:, i, h : h + 2, :],
                            rhs=e[:, h, s0:],
                            start=(pv_emitted == 0),
                            stop=(pv_emitted == n_pv - 1),
                        )
                    pv_emitted += 1

                can_trim = any(
                    kind[i][j] == "plain"
                    or (kind[i][j] == "affine" and mix_idx[i][j] == 0)
                    for i in ilist
                )

                def trim(i):
                    if can_trim and kind[i][j] == "affine":
                        return min(mix_idx[i][j], SB)
                    return 0

                # masked tiles first (their PVs are deferred so the GPSIMD
                # mask op gets the whole block); smallest trim first so the
                # first-emitted PV covers the whole block.
                ilist.sort(key=lambda i: (kind[i][j] == "plain", trim(i)))
                # plain-tile PVs are emitted one tile late so a PV waiting
                # on its exp never sits ahead of already-ready score
                # matmuls in the in-order PE stream; masked-tile PVs are
                # released once 2 further tiles have issued (the GPSIMD
                # mask is long done by then) so the block-end PV burst --
                # which delays the serial normalize chain -- stays short
                pending = []
                for it_idx, i in enumerate(ilist):
                    while deferred and deferred[0][3] <= it_idx - DEF_LAG:
                        di, de, ds0, _ = deferred.pop(0)
                        emit_pv(di, de, ds0)
                    # (bf16 scores have no minimum-moving-dim rate penalty)
                    s0 = trim(i)
                    # fp32r score matmuls drop to 1/4 rate below 256 moving
                    s0_sc = s0
                    tsl = slice(i * P, (i + 1) * P)
                    sc = scps.tile([P, 2, SB], dt.float32, tag="sc", name="sc")
                    for h in range(2):
                        hs = slice(h * 64, h * 64 + 64)
                        nc.tensor.matmul(
                            sc[:, h, s0_sc:],
                            lhsT=kh_sb[hs, hp, tsl],
                            rhs=qh_sb[hs, hp, ssl][:, s0_sc:],
                            start=True,
                            stop=True,
                        )
                            # bufs=10: affine tiles' e is held (PV deferred) until
                    # block end -- 4 held + 6 rotating in the last round
                    e = expp.tile([P, 2, SB], dt.bfloat16, tag="e", name="e", bufs=12)
                    nc.scalar.activation(
                        e[:, :, s0:],
                        sc[:, :, s0:],
                        mybir.ActivationFunctionType.Exp,
                        # fp8 weights are pre-scaled by W_SCALE on both the
                        # q and k sides; fold the (W_SCALE^2) out here
                        scale=1.0
                        / float(np.sqrt(DEPTH) * (W_SCALE**2 if FP8 else 1.0)),
                    )
                    if kind[i][j] == "affine":
                        # zero e[t', h, s'] where s' < c + t'; only the
                        # 128-col window [s0, c+128) can contain masked
                        # elements (right of it, s >= c+127 >= c+t always).
                        c = mix_idx[i][j]
                        w1 = min(SB, c + P)
                        nc.gpsimd.affine_select(
                            out=e[:, :, s0:w1],
                            in_=e[:, :, s0:w1],
                            pattern=[[0, 2], [1, w1 - s0]],
                            compare_op=mybir.AluOpType.is_ge,
                            fill=0.0,
                            base=s0 - c,
                            channel_multiplier=-1,
                        )
                        deferred.append((i, e, s0, it_idx))
                    elif kind[i][j] == "mixed":
                        u = mix_idx[i][j]
                        nc.vector.tensor_tensor(
                            e,
                            e,
                            mask_sb[:, u, None, :].to_broadcast(e.shape),
                            mybir.AluOpType.mult,
                        )
                        deferred.append((i, e, 0, it_idx))
                    else:
                        pending.append((i, e, 0))
                        if len(pending) > PV_LAG:
                            emit_pv(*pending.pop(0))
                    budget[0] += 1
                    # strict pacing: at most one unit per `cost` tiles --
                    # resetting (rather than decrementing) the credit stops
                    # accrued credit from burst-draining the queue, which
                    # left the long final rounds with no PE filler
                    if filler and budget[0] >= filler[0][1]:
                        unit, cost, _ = filler.popleft()
                        budget[0] = 0
                        unit()
                for p in pending:
                    emit_pv(*p)
                for i, e, s0, _ in deferred:
                    emit_pv(i, e, s0)
                is_final = (hp, j) == (1, N_SB - 1)
                if is_final:
                    # emit the reserved outproj units BEFORE the normalize:
                    # per-engine order is fixed at schedule time, so only
                    # work emitted here can keep PE busy through the final
                    # normalize chain
                    flush_filler_upto((1000, 0))
                # normalize: all 64 l-rows of each pv are identical; move one
                # onto the data's partitions.
                if NORM_MODE == "mmb":
                    # reciprocate one l-row of each pv straight out of psum
                    # (DVE) into the persistent staging tile, then a single
                    # K=128 selector matmul broadcasts both heads' 1/l
                    # across the data partitions
                    # fp32r keeps ~19 mantissa bits; 1/l at fp32r is far
                    # below the bf16 noise already in the probs
                    with nc.allow_low_precision(reason="1/l rounded to fp32r"):
                        nc.vector.reciprocal(ltmp_p[64:65, :], pv[0][64:65, :])
                        nc.vector.reciprocal(ltmp_p[0:1, :], pv[1][0:1, :])
                    bps = mmps.tile([P, SB], dt.float32, tag="ps", name="lb")
                    nc.tensor.matmul(
                        bps, lhsT=sel, rhs=ltmp_p,
                        start=True, stop=True,
                    )
                    # bounce 1/l to SBUF: frees the mmps slot immediately,
                    # and a TT with both operands in psum fails HW checks
                    lin = linp.tile([P, SB], dt.float32, tag="lin", name="lin")
                    nc.vector.tensor_copy(lin, bps)
                else:
                    lin = linp.tile([P, SB], dt.float32, tag="lin", name="lin")
                    ltmp = linp.tile([P, SB], dt.float32, tag="ltmp", name="ltmp")
                    nc.vector.tensor_copy(ltmp[64:128, :], pv[0][64:128, :])
                    nc.vector.tensor_copy(ltmp[0:64, :], pv[1][0:64, :])
                    nc.gpsimd.dma_start(lin[0:64, :], ltmp[64:128, :])
                    nc.gpsimd.dma_start(lin[64:128, :], ltmp[0:64, :])
                    nc.vector.reciprocal(lin, lin)
                nc.vector.tensor_tensor(
                    ao_sb[0:64, hp, ssl], pv[0][0:64, :], lin[0:64, :],
                    mybir.AluOpType.mult,
                )
                nc.vector.tensor_tensor(
                    ao_sb[64:128, hp, ssl], pv[1][64:128, :], lin[64:128, :],
                    mybir.AluOpType.mult,
                )

            def outproj_nb(nb, dgroup, kts=(0, 1), out_col0=None, tail=False):
                """Output projection for n-block nb, d-tiles [2*dgroup, +2),
                contracting feature-tiles `kts`, writing token-cols at
                out_col0 (defaults to the block's own columns).  In the
                drain phase (tail=True) psums come from the freed score
                pool (3 deep) and half the copies go to the Activation
                engine -- neither has attention work left there, and DVE
                alone was pacing the drain."""
                nsl = slice(nb * SB, (nb + 1) * SB)
                if out_col0 is None:
                    out_col0 = nb * SB
                osl = slice(out_col0, out_col0 + SB)
                ost = expp.tile([P, 2, SB], dt.bfloat16, tag="ost", name="ost", bufs=6)
                for u in range(2):
                    dtile = 2 * dgroup + u
                    if tail and u == 0:
                        # borrow the freed score ring so the drain runs a
                        # 4-deep psum pipeline instead of 2
                        ps = scps.tile([P, SB], dt.float32, tag="sc", name="po")
                    else:
                        ps = mmps.tile([P, SB], dt.float32, tag="ps", name="po")
                    for ki, kt in enumerate(kts):
                        nc.tensor.matmul(
                            ps,
                            lhsT=wo_sb[:, kt, dtile, :],
                            rhs=ao_sb[:, kt, nsl],
                            start=(ki == 0),
                            stop=(ki == len(kts) - 1),
                        )
                    # GPSIMD can't read psum; split Act/DVE in the tail
                    if tail and u == 0:
                        nc.scalar.copy(ost[:, u, :], ps)
                    else:
                        nc.vector.tensor_copy(ost[:, u, :], ps)
                if tail:
                    # the drain is dispatch-latency-bound: one DMA per
                    # dgroup, rotated across three sequencer queues
                    eng = (nc.sync, nc.scalar, nc.gpsimd)[dgroup % 3]
                    eng.dma_start(
                        outT_r[:, 2 * dgroup : 2 * dgroup + 2, osl], ost
                    )
                else:
                    nc.sync.dma_start(
                        outT_r[:, 2 * dgroup : 2 * dgroup + 2, osl], ost
                    )

            # ---- driver ----
            from collections import deque

            filler = deque()
            budget = [0]
            N_RES = int(os.environ.get("MHA_RESERVE", "3"))
            # nb2 outproj dgroups held for the drain window
            reserve = tuple(range(N_CT // 2 - N_RES, N_CT // 2))

            def push_block_units(nb):
                kh_halves = [None]
                qh_halves = [None]
                vh_halves = [None]

                def dma_unit(src, tag, store):
                    def u():
                        store[0] = load_st(src, nb, tag)
                    return u

                key = (nb, 0)
                filler.append((dma_unit(kT, "sk", kh_halves), C_DMA, key))
                filler.append((dma_unit(qT, "sq", qh_halves), C_DMA, key))
                filler.append((dma_unit(vT, "sv", vh_halves), C_DMA, key))
                def qk_units(halves_store, w_sb, dst, bn, ft):
                    state = [None]

                    def g1():
                        state[0] = project_qk_g1(halves_store[0], w_sb, ft)

                    def g2():
                        project_qk_g2(
                            halves_store[0], w_sb, dst, bn, nb, ft, state[0]
                        )

                    return g1, g2

                for ft in range(N_FT):
                    g1, g2 = qk_units(kh_halves, wk_sb, kh_sb, "k", ft)
                    filler.append((g1, C_QK - 1, key))
                    filler.append((g2, 1, key))
                for ft in range(N_FT):
                    g1, g2 = qk_units(qh_halves, wq_sb, qh_sb, "q", ft)
                    filler.append((g1, C_QK - 1, key))
                    filler.append((g2, 1, key))
                def v_units(m):
                    state = [None]

                    def g1():
                        state[0] = project_v_g1(vh_halves[0], m)

                    def g2():
                        project_v_g2(vh_halves[0], nb, m, state[0])

                    return g1, g2

                for m in range(4):
                    g1, g2 = v_units(m)
                    filler.append((g1, C_V - 1, key))
                    filler.append((g2, 1, key))

            # block (·, 0) prerequisites run serially; everything later is
            # interleaved into the attention stream.  DMA issue order
            # interleaves weight halves with token halves so the first
            # k-proj matmul (needs wk half 0 + k-tokens half 0) starts
            # ~3us in.
            def w_rearr(w):
                if FP8:
                    return w.rearrange("(ct p) (two f) -> p ct two f", p=P, two=2)
                return w.rearrange("(ct p) f -> p ct f", p=P)

            # k-path first: wk quarter 0, k quarter 0, then the rest of wk
            # in one DMA (fewer HWDGE slots), remaining k quarters; wq/wv as
            # single DMAs; wo (1MB, first needed by outproj ~25us in) is
            # deferred into the filler so block-0 v tokens land earlier.
            wkr = w_rearr(wk)
            kh0 = []

            def load_k0_quarter(h):
                if FP8:
                    sth = stA.tile([P, 2, 2, SB], IN_DT, tag=f"sk{h}", name=f"sk{h}")
                    nc.sync.dma_start(
                        sth,
                        kT.rearrange("(ct p) n2 -> p ct n2", p=P)[
                            :, 2 * h : 2 * h + 2, 0 : 2 * SB
                        ],
                    )
                else:
                    sth = stA.tile([P, 2, SB], IN_DT, tag=f"sk{h}", name=f"sk{h}")
                    nc.sync.dma_start(
                        sth,
                        kT[:, 0:SB].rearrange("(ct p) n -> p ct n", p=P)[
                            :, 2 * h : 2 * h + 2
                        ],
                    )
                kh0.append(sth)

            nc.sync.dma_start(wk_sb[:, 0:2], wkr[:, 0:2])
            load_k0_quarter(0)
            nc.sync.dma_start(wk_sb[:, 2:8], wkr[:, 2:8])
            for h in range(1, 4):
                load_k0_quarter(h)
            nc.sync.dma_start(wq_sb, w_rearr(wq))
            qh0 = load_st(qT, 0, "sq")
            nc.sync.dma_start(wv_sb, w_rearr(wv))
            vh0 = load_st(vT, 0, "sv")
            if n_patterns:
                nc.sync.dma_start(mask_sb, maskt.rearrange("m p s -> p m s"))
            if has_bias:
                nc.sync.dma_start(
                    bias_sb["q"], bq.rearrange("(ft p) o -> p (ft o)", p=P)
                )
                nc.sync.dma_start(
                    bias_sb["k"], bk.rearrange("(ft p) o -> p (ft o)", p=P)
                )
            for ft in range(N_FT):
                project_qk(kh0, wk_sb, kh_sb, "k", 0, ft)
            for ft in range(N_FT):
                project_qk(qh0, wq_sb, qh_sb, "q", 0, ft)
            # j=0 scores need only kh/qh: emitting them here (before the
            # v-projections in the PE stream) fills the wait for the
            # block-0 v-token DMAs with useful score+exp work; the PVs in
            # attention_block consume them as phase-A records.
            PRO_HOIST = int(os.environ.get("MHA_PRO_HOIST", "0"))
            if PRO_HOIST:
                for php in range(2):
                    ilist0, trim0 = block_order(php, 0)
                    for i in ilist0[:PRO_HOIST]:
                        hoisted.setdefault((php, 0), []).append(
                            emit_score_exp(php, 0, i, trim0)
                        )
            for m in range(4):
                project_v(vh0, 0, m)
            def push_hoist_units(j):
                for hp in range(2):
                    ilist, trim = block_order(hp, j)

                    for i in ilist[:HOIST]:
                        filler.append(
                            (
                                lambda hp=hp, j=j, i=i, trim=trim: hoisted.setdefault(
                                    (hp, j), []
                                ).append(emit_score_exp(hp, j, i, trim)),
                                C_HS,
                                (j, hp),
                            )
                        )

            start_b3 = 0
            for nb in range(1, N_NB):
                if nb == N_NB - 1:
                    start_b3 = len(filler)
                push_block_units(nb)
                if nb >= int(os.environ.get("MHA_HOIST_MIN_J", "2")):
                    push_hoist_units(nb)

            def load_wo():
                nc.sync.dma_start(
                    wo_sb, wo.rearrange("(kt p) (dt q) -> p kt dt q", p=P, q=P)
                )

            # after block-1's input DMAs (k1 feeds j=1 scores), before its
            # projection units; forced by the start of j=1 at the latest
            filler.insert(start_b3 + 2, (load_wo, C_DMA, (1, 0)))
            last = N_SB - 1
            for j in range(N_SB):
                attention_block(0, j)
                if j == last:
                    # kt=0 half of the last block's outproj runs as filler
                    # during the final attention block; kt=1 follows after
                    # (host sums the two partials).  Two nb2 units are held
                    # in reserve behind them (cost 99 blocks budget-drain)
                    # to keep PE hot through the final normalize chain.
                    C_KT0 = int(os.environ.get("MHA_C_KT0", "1"))
                    for dg in range(N_CT // 2):
                        filler.append(
                            (lambda g=dg: outproj_nb(last, g, kts=(0,)), C_KT0, None)
                        )
                    for g in reserve:
                        # the key is never force-flushed and the huge cost
                        # is never budget-drained -- these pop only in the
                        # final drain loop, after the last normalize is
                        # emitted, keeping PE hot through that chain
                        filler.append(
                            (
                                lambda x=g: outproj_nb(last - 1, x, tail=True),
                                10**9,
                                (999, 0),
                            )
                        )
                attention_block(1, j)
                if j == last:
                    for dg in range(N_CT // 2):
                        filler.append(
                            (
                                lambda g=dg: outproj_nb(
                                    last, g, kts=(1,), out_col0=NTOK_LOC,
                                    tail=True,
                                ),
                                1,
                                None,
                            )
                        )
                else:
                    dgs = range(N_CT // 2)
                    if j == last - 1:
                        dgs = range(N_CT // 2 - len(reserve))
                    for dg in dgs:
                        # higher cost spreads outproj into the
                        # (filler-starved) late attention rounds
                        filler.append((lambda x=j, g=dg: outproj_nb(x, g), C_OP, None))
            while filler:
                filler.popleft()[0]()

    nc.compile()
    return nc


_NC_CACHE = {}


def _get_nc(kind_key, kind, mix_idx, n_patterns, has_bias):
    key = (kind_key, n_patterns, has_bias)
    if key not in _NC_CACHE:
        _NC_CACHE[key] = _build_nc(kind, mix_idx, n_patterns, has_bias)
    return _NC_CACHE[key]


F8 = ml_dtypes.float8_e4m3


def _split8(x):
    """fp32 [D, N] -> [D, 2N] fp8e4 with (hi, lo) interleaved per
    512-token block: layout [D, nb, {hi, lo}, SB]."""
    d = x.shape[0]
    hi = x.astype(F8)
    lo = (x - hi.astype(np.float32)).astype(F8)
    a = np.stack(
        [hi.reshape(d, -1, SB), lo.reshape(d, -1, SB)], axis=2
    )  # [D, nb, 2, SB]
    return np.ascontiguousarray(a.reshape(d, -1))


def _splitw8(wT):
    """Weight [D, F] fp32, pre-scaled: -> [D, 2F] fp8 in (lo, hi) order."""
    ws = wT * W_SCALE
    hi = ws.astype(F8)
    lo = (ws - hi.astype(np.float32)).astype(F8)
    return np.ascontiguousarray(
        np.stack([lo, hi], axis=1).reshape(wT.shape[0], -1)
    )


def kernel(v, k, q, mask, Wq, bq, Wk, bk, Wv, bv, Wo, bo, trace=False):
    global LAST_RESULTS, LAST_EXEC_WALL
    from concourse.bass_utils import run_bass_kernel_spmd

    in_np = ml_dtypes.bfloat16

    def prep_T(x):  # [S, D] -> [D, S] in input dtype (or fp8 hi/lo pair)
        xT = np.ascontiguousarray(np.asarray(x, dtype=np.float32).T)
        if FP8:
            return _split8(xT)
        return xT.astype(in_np)

    kind, mix_idx, patterns = _mask_structure(np.asarray(mask, dtype=np.float32))
    maskt = (
        np.ascontiguousarray(np.stack(patterns)).astype(ml_dtypes.bfloat16)
        if patterns
        else None
    )

    has_bias = bool(np.any(np.asarray(bq)) or np.any(np.asarray(bk)))
    kind_key = str(kind) + str(mix_idx)
    nc = _get_nc(kind_key, kind, mix_idx, len(patterns), has_bias)

    q_np = np.asarray(q, np.float32)
    k_np = np.asarray(k, np.float32)
    v_np = np.asarray(v, np.float32)
    qT = [prep_T(q_np[b]) for b in range(BATCH)]
    kT = [prep_T(k_np[b]) for b in range(BATCH)]
    vT = [prep_T(v_np[b]) for b in range(BATCH)]

    in_maps = []
    for core in range(N_CORES):
        b = core // 4
        hg = core % 4
        fsl = slice(hg * FW, (hg + 1) * FW)
        def prep_w(W):
            wT = np.ascontiguousarray(np.asarray(W, np.float32)[fsl].T)
            if FP8:
                return _splitw8(wT)
            return wT.astype(in_np)

        wo_scale = W_SCALE if FP8 else 1.0
        m = {
            "qT": qT[b],
            "kT": kT[b],
            "vT": vT[b],
            "wqT": prep_w(Wq),
            "wkT": prep_w(Wk),
            "wvT": prep_w(Wv),
            "woT": np.ascontiguousarray(
                np.asarray(Wo, np.float32)[:, fsl].T / wo_scale
            ),
        }
        if maskt is not None:
            m["maskt"] = maskt
        if has_bias:
            # projections are scaled by W_SCALE in fp8 mode; scale the
            # biases to match (the exp scale folds it back out)
            m["bq"] = np.asarray(bq, np.float32)[fsl].reshape(FW, 1) * wo_scale
            m["bk"] = np.asarray(bk, np.float32)[fsl].reshape(FW, 1) * wo_scale
        in_maps.append(m)

    import time as _time

    _t0 = _time.time()
    res = run_bass_kernel_spmd(
        nc, in_maps, core_ids=list(range(N_CORES)), trace=trace
    )
    LAST_EXEC_WALL = _time.time() - _t0
    LAST_RESULTS = res

    out = np.zeros((BATCH, SEQ, D_MODEL), dtype=np.float32)
    last0 = (N_NB - 1) * SB
    for core in range(N_CORES):
        b = core // 4
        oT = res.results[core]["outT"]
        out[b] += oT[:, :NTOK_LOC].T
        # kt=1 partial of the last n-block lives in the extra columns
        out[b, last0 : last0 + SB] += oT[:, NTOK_LOC:].T
    # v-bias contributes the constant bv @ Wo.T (softmax rows sum to 1)
    out += (
        np.asarray(bo, np.float32)
        + np.asarray(bv, np.float32) @ np.asarray(Wo, np.float32).T
    )[None, None, :]
    return out



# revision 53
# speedup vs baseline: 1.0065x; 1.0065x over previous
"""Multi-head attention (B=2, S=2048, D=1024, H=16) on 8 Trainium2 cores.

Sharding: 2-way data parallel over batch x 4-way tensor parallel over heads.
Each core owns one batch and 4 heads (a 256-feature slice).  Per core:
  - QKV projections for its feature slice over its batch's 2048 tokens
  - causal attention for its 4 heads with block-skipping
  - partial output projection (contraction over its 256 features)
Host: transposes/splits inputs, sums the 4 partial outputs per batch, adds
bo (+ Wo@bv: softmax rows sum to 1, so the v bias is a constant shift).

On-chip pipeline:
  - QKV projections run as fp8e4 DoubleRow matmuls (0.5 cycles/row) with a
    hi+lo 3-term split (x8*w8 + x8*wr + xr*w8) prepared on the host;
    weights pre-scaled x32 to keep residuals out of the fp8 subnormal
    range.  0.75x the bf16 PE time at ~2x BETTER accuracy.
  - v is projected DIRECTLY into token-major layout (lhsT = the v input
    tile, moving dim = features) -- no PE transposes.
  - scores run float32r at full PE rate on fp32r qh/kh (moving >= 256);
    probs/v are bf16 (fp8 would blow the 2e-2 error budget).
  - causal masking: GPSIMD affine_select on just the 128-column window
    straddling the diagonal; fully-masked tiles skipped, trims applied.
  - softmax row-sums come free from ones-columns in the PV lhsT; 1/l is
    broadcast across partitions by K=1 PE matmuls against a one-row
    selector (GPSIMD cannot read psum; partition_broadcast is broken).
  - masked-tile PVs release 2 tiles late, plain-tile PVs 1 tile late, so
    nothing not-yet-ready ever blocks ready score matmuls in the in-order
    PE stream.
Cross-phase work (next-block projections + DMAs, finished-block output
projections) is interleaved into the attention stream with strict budget
pacing; the last n-block's output projection is split by feature-half so
one half runs during the final attention block, and reserved outproj
units are emitted just before the final normalize to keep PE hot through
the drain.  Startup: the k-path DMAs go first (wk quarter 0, k quarter 0,
rest of wk merged), wq/wv load as single DMAs, and the 1MB wo transfer is
deferred into the filler stream (slot 40, among block-3's input DMAs) so
block-0/1/2 input quarters are never queued behind it on the DMA engine.
PV emission trails the score/exp stream by PV_LAG=4 tiles (deferred
masked tiles by DEF_LAG=4): the ACT engine -- which saturates in the
causally-backloaded second half -- gets a 4-tile exp backlog cushion, so
PV matmuls almost never stall the in-order PE stream on a pending exp.
The final block runs lag 2/2 instead so its PV burst doesn't push the
drain's normalize chain later.  (A score-hoisting mechanism exists behind
MHA_HOIST -- emitting future blocks' score+exp as low-priority filler --
but every setting regressed: a hoisted score pins one of the two psum
score-ring slots until its exp drains through the ACT backlog, which in
the ACT-saturated second half stalls the in-order PE stream more than the
overlap gains.  Default 0.)
The softmax 1/l broadcast is a single K=128 selector matmul per block
(dead selector/staging rows zeroed once at startup; the staging tile is
persistent so the zeros survive all blocks).
Validated: 129657 ns (TimelineSim), rel err 4.6e-3 on HW.
"""

import os

import numpy as np
import ml_dtypes

D_MODEL = 1024
NUM_HEADS = 16
DEPTH = 64
BATCH = 2
SEQ = 2048
N_CORES = 8
H_LOC = 4  # heads per core
FW = 256  # features per core (4 heads x 64)
P = 128
SB = 512  # s-block width
NTOK_LOC = SEQ  # tokens per core (one batch)
N_SB = SEQ // SB  # 4 s-blocks
N_TT = SEQ // P  # 16 t-tiles
N_NB = NTOK_LOC // SB  # 4 n-blocks
N_CT = D_MODEL // P  # 8 contraction tiles
N_FT = FW // P  # 2 feature tiles
N_HP = H_LOC // 2  # 2 head-pairs

NORM_MODE = os.environ.get("MHA_NORM", "mmb")  # mmb | dma
# fp8 3-term projections: inputs/weights are split hi+lo into fp8e4 on the
# host; each 128-deep contraction tile needs 3 of the 4 cross products
# (x8*w8, x8*wr, xr*w8 -- the dropped xr*wr term is O(quant_err^2)), and
# DoubleRow packs 2 products per matmul at 0.5 cycles/row: 12 instructions
# replace 8 at 0.75x the PE time with ~2x BETTER accuracy than bf16 inputs.
# Weights are pre-scaled by W_SCALE so their residuals stay out of the
# fp8 subnormal range; the scale is folded into the exp scale and Wo.
FP8 = os.environ.get("MHA_FP8", "1") == "1"
W_SCALE = 32.0
# filler pacing costs (in score-tile-iteration credits)
C_DMA = int(os.environ.get("MHA_C_DMA", "1"))
C_QK = int(os.environ.get("MHA_C_QK", "2"))
C_V = int(os.environ.get("MHA_C_V", "2"))
C_OP = int(os.environ.get("MHA_C_OP", "2"))
PV_LAG = int(os.environ.get("MHA_PV_LAG", "4"))
DEF_LAG = int(os.environ.get("MHA_DEF_LAG", "4"))

LAST_RESULTS = None
LAST_EXEC_WALL = None


def _mask_structure(mask_np):
    """Classify each (t-tile, s-block) of the [S, S] mask (1.0 = disallowed).

    Returns (kind, mix_idx, patterns): kind[i][j] in
    {"skip", "plain", "affine", "mixed"}; for "affine", mix_idx[i][j] is the
    offset c of keep = (s >= c + t); for "mixed" it indexes into patterns
    (list of [P, SB] keep-masks).  mask rows = query s, cols = key t;
    scoresT is [t, s] so we transpose.
    """
    maskT = np.ascontiguousarray(mask_np.reshape(SEQ, SEQ).T)
    kind = [[None] * N_SB for _ in range(N_TT)]
    mix_idx = [[None] * N_SB for _ in range(N_TT)]
    patterns = []
    pat_key = {}
    s_idx = np.arange(SB)[None, :]
    t_idx = np.arange(P)[:, None]
    for i in range(N_TT):
        for j in range(N_SB):
            sub = maskT[i * P : (i + 1) * P, j * SB : (j + 1) * SB]
            if np.all(sub >= 0.5):
                kind[i][j] = "skip"
                continue
            if np.all(sub < 0.5):
                kind[i][j] = "plain"
                continue
            keep = (sub < 0.5).astype(np.float32)
            first_one = np.argmax(keep, axis=1)
            c = int(first_one[0])
            if np.array_equal(keep, (s_idx >= c + t_idx).astype(np.float32)):
                kind[i][j] = "affine"
                mix_idx[i][j] = c
                continue
            kind[i][j] = "mixed"
            key = keep.tobytes()
            if key not in pat_key:
                pat_key[key] = len(patterns)
                patterns.append(keep)
            mix_idx[i][j] = pat_key[key]
    return kind, mix_idx, patterns


def _build_nc(kind, mix_idx, n_patterns, has_bias):
    import concourse.tile as tile
    import concourse.mybir as mybir
    from concourse import bacc

    dt = mybir.dt

    nc = bacc.Bacc(None, target_bir_lowering=False)

    if FP8:
        # inputs ship as (hi, lo) fp8e4 pairs interleaved per 512-token
        # block (so DMA slices stay <= 3 dims); weights interleave
        # (lo, hi) at full width.  The orders make the DoubleRow slot APs
        # of all three product groups regular-stride slices.
        IN_DT = dt.float8e4
        in_shape = [D_MODEL, 2 * NTOK_LOC]
        w_shape = [D_MODEL, 2 * FW]
    else:
        IN_DT = dt.bfloat16
        in_shape = [D_MODEL, NTOK_LOC]
        w_shape = [D_MODEL, FW]
    qT = nc.dram_tensor("qT", in_shape, IN_DT, kind="ExternalInput")
    kT = nc.dram_tensor("kT", in_shape, IN_DT, kind="ExternalInput")
    vT = nc.dram_tensor("vT", in_shape, IN_DT, kind="ExternalInput")
    wq = nc.dram_tensor("wqT", w_shape, IN_DT, kind="ExternalInput")
    wk = nc.dram_tensor("wkT", w_shape, IN_DT, kind="ExternalInput")
    wv = nc.dram_tensor("wvT", w_shape, IN_DT, kind="ExternalInput")
    wo = nc.dram_tensor("woT", [FW, D_MODEL], dt.float32r, kind="ExternalInput")
    maskt = None
    if n_patterns:
        maskt = nc.dram_tensor(
            "maskt", [n_patterns, P, SB], dt.bfloat16, kind="ExternalInput"
        )
    bq = bk = bv = None
    if has_bias:
        bq = nc.dram_tensor("bq", [FW, 1], dt.float32, kind="ExternalInput")
        bk = nc.dram_tensor("bk", [FW, 1], dt.float32, kind="ExternalInput")
        bv = nc.dram_tensor("bv", [FW, 1], dt.float32, kind="ExternalInput")
    # cols [NTOK_LOC, NTOK_LOC+SB) hold the kt=1 partial of the last
    # n-block (its outproj is split by feature-half so the first half can
    # run during the final attention block); host sums the two partials.
    outT = nc.dram_tensor(
        "outT", [D_MODEL, NTOK_LOC + SB], dt.bfloat16, kind="ExternalOutput"
    )
    outT_r = outT.rearrange("(dt p) n -> p dt n", p=P)

    with tile.TileContext(nc) as tc:
        with (
            tc.tile_pool(name="const", bufs=1) as const,
            tc.tile_pool(name="big", bufs=1) as big,
            tc.tile_pool(name="stageA", bufs=2) as stA,
            tc.tile_pool(name="expp", bufs=6) as expp,
            tc.tile_pool(name="linp", bufs=2) as linp,
            # PSUM: mm 2x1 banks + sc 2x2 banks + pv 2x1 = 8 banks
            tc.tile_pool(name="mmps", bufs=2, space="PSUM") as mmps,
            tc.tile_pool(name="scps", bufs=2, space="PSUM") as scps,
            tc.tile_pool(name="pvps", bufs=1, space="PSUM") as pvps,
        ):
            # ---- constants (tiles only; DMAs are interleaved with the
            # first token-block loads in the driver so the first k-proj
            # matmul is not queued behind 6us of weight transfers) ----
            w_sb_shape = [P, N_CT, 2, FW] if FP8 else [P, N_CT, FW]
            wq_sb = const.tile(w_sb_shape, IN_DT)
            wk_sb = const.tile(w_sb_shape, IN_DT)
            wv_sb = const.tile(w_sb_shape, IN_DT)
            wo_sb = const.tile([P, N_FT, N_CT, P], dt.float32r)
            mask_sb = None
            if n_patterns:
                mask_sb = const.tile([P, n_patterns, SB], dt.bfloat16)
            bias_sb = {}
            if has_bias:
                for name in ("q", "k"):
                    bias_sb[name] = const.tile(
                        [P, N_FT], dt.float32, name=f"b{name}"
                    )

            # ---- persistent activations ----
            # bf16 qh/kh run the score matmuls at the same PE rate but
            # without fp32r's >=256-moving-dim constraint, so diagonal
            # tiles trim fully; the ~0.2% quantization is far under budget
            qh_sb = big.tile([P, N_FT, NTOK_LOC], dt.bfloat16)
            kh_sb = big.tile([P, N_FT, NTOK_LOC], dt.bfloat16)
            ao_sb = big.tile([P, N_FT, NTOK_LOC], dt.float32r)
            # selector for the l-broadcast matmul: sel[64, 0:64]=1 routes
            # ltmp row 64 (1/l_a) -> psum rows 0:64; sel[0, 64:128]=1
            # routes row 0 (1/l_b) -> rows 64:128.  ONE K=65 matmul (cost
            # is moving-dim only: 213ns) replaces two K=1 matmuls; all
            # dead rows of sel AND of both ltmp ring buffers are zeroed
            # once at startup so the full-128 contraction (partial ranges
            # fail the BIR verifier) contributes exact zeros elsewhere.
            sel = const.tile([P, P], dt.float32r, name="sel")
            if NORM_MODE == "mmb":
                # memset can't emit fp32r: memset an fp32 scratch, then a
                # DVE copy (a legal fp32r producer) rounds it over
                selt = const.tile([P, P], dt.float32, name="selt")
                nc.vector.memset(selt, 0.0)
                nc.vector.memset(selt[64:65, 0:64], 1.0)
                nc.vector.memset(selt[0:1, 64:128], 1.0)
                nc.vector.tensor_copy(sel, selt)
                # persistent 1/l staging row-pair: one allocation, dead
                # rows zeroed once; each block's recips overwrite only
                # rows 64 (1/l_a) and 0 (1/l_b)
                ltmp_p = big.tile([P, SB], dt.float32r, name="ltmp_p")
                zscr = const.tile([P, SB], dt.float32, name="zscr")
                nc.vector.memset(zscr, 0.0)
                # partition slices must start at 0/32/64: zero the whole
                # tile; the per-block recips then own rows 0 and 64
                nc.vector.tensor_copy(ltmp_p, zscr)
            # per head-pair: [t', t-tile, group, 64] bf16 with groups
            # [vh_a | ones | vh_b]:
            #   h0: lhsT = groups 0:2 = [v_a | 1] -> psum [data(0:64); l(64:128)]
            #   h1: lhsT = groups 1:3 = [1 | v_b] -> psum [l(0:64); data(64:128)]
            # (the 4D shape lets one strided copy fill both data groups)
            vh_sb = [
                big.tile([P, N_TT, 3, 64], dt.bfloat16, name=f"vh{hp}")
                for hp in range(N_HP)
            ]
            for hp in range(N_HP):
                nc.vector.memset(vh_sb[hp][:, :, 1, :], 1.0)

            def copyback(dst_ap, ps, bias_ap):
                if bias_ap is not None:
                    nc.vector.tensor_tensor(
                        dst_ap, ps, bias_ap.to_broadcast(ps.shape),
                        mybir.AluOpType.add,
                    )
                else:
                    nc.vector.tensor_copy(dst_ap, ps)

            def load_st(src, nb, tag):
                """DMA one 512-token block of one input, in four quarters
                (the ct-loop consumer starts after the first lands)."""
                nsl = slice(nb * SB, (nb + 1) * SB)
                quarters = []
                for h in range(4):
                    if FP8:
                        sth = stA.tile(
                            [P, 2, 2, SB], IN_DT, tag=f"{tag}{h}", name=f"{tag}{h}"
                        )
                        nc.sync.dma_start(
                            sth,
                            src.rearrange("(ct p) n2 -> p ct n2", p=P)[
                                :,
                                2 * h : 2 * h + 2,
                                nb * 2 * SB : (nb + 1) * 2 * SB,
                            ],
                        )
                    else:
                        sth = stA.tile(
                            [P, 2, SB], IN_DT, tag=f"{tag}{h}", name=f"{tag}{h}"
                        )
                        nc.sync.dma_start(
                            sth,
                            src[:, nsl].rearrange("(ct p) n -> p ct n", p=P)[
                                :, 2 * h : 2 * h + 2, :
                            ],
                        )
                    quarters.append(sth)
                return quarters

            DR = mybir.MatmulPerfMode.DoubleRow

            def project_qk_g1(halves, w_sb, ft):
                fsl = slice(ft * P, (ft + 1) * P)
                ps = mmps.tile([P, SB], dt.float32, tag="ps", name="ps")
                for h in range(4):
                    nc.tensor.matmul(
                        ps,
                        lhsT=w_sb[:, 2 * h : 2 * h + 2, 1, fsl],
                        rhs=halves[h][:, :, 0, :],
                        start=(h == 0),
                        stop=False,
                        perf_mode=DR,
                    )
                return ps

            def project_qk_g2(halves, w_sb, dst, bn, nb, ft, ps):
                fsl = slice(ft * P, (ft + 1) * P)
                for ct in range(N_CT):
                    nc.tensor.matmul(
                        ps,
                        lhsT=w_sb[:, ct, :, fsl],
                        rhs=halves[ct // 2][:, ct % 2, :, :],
                        start=False,
                        stop=(ct == N_CT - 1),
                        perf_mode=DR,
                    )
                bias_ap = (
                    bias_sb[bn][:, ft : ft + 1] if (has_bias and bn) else None
                )
                copyback(dst[:, ft, nb * SB : (nb + 1) * SB], ps, bias_ap)

            def project_qk(halves, w_sb, dst, bn, nb, ft):
                fsl = slice(ft * P, (ft + 1) * P)
                ps = mmps.tile([P, SB], dt.float32, tag="ps", name="ps")
                if FP8:
                    # G1: x8*w8 paired over each quarter's 2 k-tiles
                    for h in range(4):
                        nc.tensor.matmul(
                            ps,
                            lhsT=w_sb[:, 2 * h : 2 * h + 2, 1, fsl],
                            rhs=halves[h][:, :, 0, :],
                            start=(h == 0),
                            stop=False,
                            perf_mode=DR,
                        )
                    # G2: (wr, w8) x (x8, xr) per k-tile = x8*wr + xr*w8
                    for ct in range(N_CT):
                        nc.tensor.matmul(
                            ps,
                            lhsT=w_sb[:, ct, :, fsl],
                            rhs=halves[ct // 2][:, ct % 2, :, :],
                            start=False,
                            stop=(ct == N_CT - 1),
                            perf_mode=DR,
                        )
                else:
                    for ct in range(N_CT):
                        nc.tensor.matmul(
                            ps,
                            lhsT=w_sb[:, ct, fsl],
                            rhs=halves[ct // 2][:, ct % 2, :],
                            start=(ct == 0),
                            stop=(ct == N_CT - 1),
                        )
                bias_ap = (
                    bias_sb[bn][:, ft : ft + 1] if (has_bias and bn) else None
                )
                copyback(dst[:, ft, nb * SB : (nb + 1) * SB], ps, bias_ap)

            def project_v_g1(halves, m):
                msl = slice(m * P, (m + 1) * P)
                ps = mmps.tile([P, N_HP, 2, 64], dt.float32, tag="ps", name="psv")
                for h in range(4):
                    nc.tensor.matmul(
                        ps,
                        lhsT=halves[h][:, :, 0, msl],
                        rhs=wv_sb[:, 2 * h : 2 * h + 2, 1, :],
                        start=(h == 0),
                        stop=False,
                        perf_mode=DR,
                    )
                return ps

            def project_v_g2(halves, nb, m, ps):
                tt = nb * 4 + m
                msl = slice(m * P, (m + 1) * P)
                for ct in range(N_CT):
                    nc.tensor.matmul(
                        ps,
                        lhsT=halves[ct // 2][:, ct % 2, :, msl],
                        rhs=wv_sb[:, ct, :, :],
                        start=False,
                        stop=(ct == N_CT - 1),
                        perf_mode=DR,
                    )
                for hp in range(N_HP):
                    nc.vector.tensor_copy(
                        vh_sb[hp][:, tt, 0:3:2, :], ps[:, hp, :, :]
                    )

            def project_v(halves, nb, m):
                """Project tokens [nb*SB + m*P, +P) of v directly into the
                token-major vh tiles (lhsT = the v input tile; moving dim =
                the 256 output features)."""
                tt = nb * 4 + m
                msl = slice(m * P, (m + 1) * P)
                ps = mmps.tile([P, N_HP, 2, 64], dt.float32, tag="ps", name="psv")
                if FP8:
                    for h in range(4):
                        nc.tensor.matmul(
                            ps,
                            lhsT=halves[h][:, :, 0, msl],
                            rhs=wv_sb[:, 2 * h : 2 * h + 2, 1, :],
                            start=(h == 0),
                            stop=False,
                            perf_mode=DR,
                        )
                    for ct in range(N_CT):
                        nc.tensor.matmul(
                            ps,
                            lhsT=halves[ct // 2][:, ct % 2, :, msl],
                            rhs=wv_sb[:, ct, :, :],
                            start=False,
                            stop=(ct == N_CT - 1),
                            perf_mode=DR,
                        )
                else:
                    for ct in range(N_CT):
                        nc.tensor.matmul(
                            ps,
                            lhsT=halves[ct // 2][:, ct % 2, msl],
                            rhs=wv_sb[:, ct, :],
                            start=(ct == 0),
                            stop=(ct == N_CT - 1),
                        )
                # v bias is folded on the host: softmax rows sum to 1, so
                # bv contributes the constant bv @ Wo.T to the output.
                # One strided copy per head-pair fills both data groups
                # (dest groups {0, 2} via a step-2 slice).
                for hp in range(N_HP):
                    nc.vector.tensor_copy(
                        vh_sb[hp][:, tt, 0:3:2, :], ps[:, hp, :, :]
                    )

            def flush_filler_upto(key):
                while filler and (filler[0][2] is None or filler[0][2] <= key):
                    unit, cost, _ = filler.popleft()
                    unit()

            def attention_block(hp, j):
                flush_filler_upto((j, hp))
                ilist = [i for i in range(N_TT) if kind[i][j] != "skip"]
                assert ilist, "fully-masked s-block unsupported"
                pv = [
                    pvps.tile([P, SB], dt.float32, tag=f"pv{h}", name=f"pv{h}")
                    for h in range(2)
                ]
                ssl = slice(j * SB, (j + 1) * SB)
                n_pv = len(ilist)
                pv_emitted = 0
                deferred = []

                def emit_pv(i, e, s0):
                    nonlocal pv_emitted
                    for h in range(2):
                        nc.tensor.matmul(
                            pv[h][:, s0:],
                            lhsT=vh_sb[hp][:, i, h : h + 2, :],
                            rhs=e[:, h, s0:],
                            start=(pv_emitted == 0),
                            stop=(pv_emitted == n_pv - 1),
                        )
                    pv_emitted += 1

                can_trim = any(
                    kind[i][j] == "plain"
                    or (kind[i][j] == "affine" and mix_idx[i][j] == 0)
                    for i in ilist
                )

                def trim(i):
                    if can_trim and kind[i][j] == "affine":
                        return min(mix_idx[i][j], SB)
                    return 0

                # masked tiles first (their PVs are deferred so the GPSIMD
                # mask op gets the whole block); smallest trim first so the
                # first-emitted PV covers the whole block.
                ilist.sort(key=lambda i: (kind[i][j] == "plain", trim(i)))
                # plain-tile PVs are emitted one tile late so a PV waiting
                # on its exp never sits ahead of already-ready score
                # matmuls in the in-order PE stream; masked-tile PVs are
                # released once 2 further tiles have issued (the GPSIMD
                # mask is long done by then) so the block-end PV burst --
                # which delays the serial normalize chain -- stays short
                pending = []
                for it_idx, i in enumerate(ilist):
                    while deferred and deferred[0][3] <= it_idx - DEF_LAG:
                        di, de, ds0, _ = deferred.pop(0)
                        emit_pv(di, de, ds0)
                    # (bf16 scores have no minimum-moving-dim rate penalty)
                    s0 = trim(i)
                    # fp32r score matmuls drop to 1/4 rate below 256 moving
                    s0_sc = s0
                    tsl = slice(i * P, (i + 1) * P)
                    sc = scps.tile([P, 2, SB], dt.float32, tag="sc", name="sc")
                    for h in range(2):
                        hs = slice(h * 64, h * 64 + 64)
                        nc.tensor.matmul(
                            sc[:, h, s0_sc:],
                            lhsT=kh_sb[hs, hp, tsl],
                            rhs=qh_sb[hs, hp, ssl][:, s0_sc:],
                            start=True,
                            stop=True,
                        )
                            # bufs=10: affine tiles' e is held (PV deferred) until
                    # block end -- 4 held + 6 rotating in the last round
                    e = expp.tile([P, 2, SB], dt.bfloat16, tag="e", name="e", bufs=12)
                    nc.scalar.activation(
                        e[:, :, s0:],
                        sc[:, :, s0:],
                        mybir.ActivationFunctionType.Exp,
                        # fp8 weights are pre-scaled by W_SCALE on both the
                        # q and k sides; fold the (W_SCALE^2) out here
                        scale=1.0
                        / float(np.sqrt(DEPTH) * (W_SCALE**2 if FP8 else 1.0)),
                    )
                    if kind[i][j] == "affine":
                        # zero e[t', h, s'] where s' < c + t'; only the
                        # 128-col window [s0, c+128) can contain masked
                        # elements (right of it, s >= c+127 >= c+t always).
                        c = mix_idx[i][j]
                        w1 = min(SB, c + P)
                        nc.gpsimd.affine_select(
                            out=e[:, :, s0:w1],
                            in_=e[:, :, s0:w1],
                            pattern=[[0, 2], [1, w1 - s0]],
                            compare_op=mybir.AluOpType.is_ge,
                            fill=0.0,
                            base=s0 - c,
                            channel_multiplier=-1,
                        )
                        deferred.append((i, e, s0, it_idx))
                    elif kind[i][j] == "mixed":
                        u = mix_idx[i][j]
                        nc.vector.tensor_tensor(
                            e,
                            e,
                            mask_sb[:, u, None, :].to_broadcast(e.shape),
                            mybir.AluOpType.mult,
                        )
                        deferred.append((i, e, 0, it_idx))
                    else:
                        pending.append((i, e, 0))
                        if len(pending) > PV_LAG:
                            emit_pv(*pending.pop(0))
                    budget[0] += 1
                    # strict pacing: at most one unit per `cost` tiles --
                    # resetting (rather than decrementing) the credit stops
                    # accrued credit from burst-draining the queue, which
                    # left the long final rounds with no PE filler
                    if filler and budget[0] >= filler[0][1]:
                        unit, cost, _ = filler.popleft()
                        budget[0] = 0
                        unit()
                for p in pending:
                    emit_pv(*p)
                for i, e, s0, _ in deferred:
                    emit_pv(i, e, s0)
                is_final = (hp, j) == (1, N_SB - 1)
                if not is_final:
                    # the normalize's lb matmul waits ~2x658ns of DVE recips
                    # right after the last PV; force a filler unit into that
                    # window (budget pacing only runs inside the tile loop)
                    for _ in range(NORM_POPS):
                        if filler:
                            unit, _c, _k = filler.popleft()
                            budget[0] = 0
                            unit()
                if is_final:
                    # emit the reserved outproj units BEFORE the normalize:
                    # per-engine order is fixed at schedule time, so only
                    # work emitted here can keep PE busy through the final
                    # normalize chain
                    flush_filler_upto((1000, 0))
                # normalize: all 64 l-rows of each pv are identical; move one
                # onto the data's partitions.
                if NORM_MODE == "mmb":
                    # reciprocate one l-row of each pv straight out of psum
                    # (DVE) into the persistent staging tile, then a single
                    # K=128 selector matmul broadcasts both heads' 1/l
                    # across the data partitions
                    # fp32r keeps ~19 mantissa bits; 1/l at fp32r is far
                    # below the bf16 noise already in the probs
                    with nc.allow_low_precision(reason="1/l rounded to fp32r"):
                        nc.vector.reciprocal(ltmp_p[64:65, :], pv[0][64:65, :])
                        nc.vector.reciprocal(ltmp_p[0:1, :], pv[1][0:1, :])
                    bps = mmps.tile([P, SB], dt.float32, tag="ps", name="lb")
                    nc.tensor.matmul(
                        bps, lhsT=sel, rhs=ltmp_p,
                        start=True, stop=True,
                    )
                    # bounce 1/l to SBUF: frees the mmps slot immediately,
                    # and a TT with both operands in psum fails HW checks
                    lin = linp.tile([P, SB], dt.float32, tag="lin", name="lin")
                    nc.vector.tensor_copy(lin, bps)
                else:
                    lin = linp.tile([P, SB], dt.float32, tag="lin", name="lin")
                    ltmp = linp.tile([P, SB], dt.float32, tag="ltmp", name="ltmp")
                    nc.vector.tensor_copy(ltmp[64:128, :], pv[0][64:128, :])
                    nc.vector.tensor_copy(ltmp[0:64, :], pv[1][0:64, :])
                    nc.gpsimd.dma_start(lin[0:64, :], ltmp[64:128, :])
                    nc.gpsimd.dma_start(lin[64:128, :], ltmp[0:64, :])
                    nc.vector.reciprocal(lin, lin)
                nc.vector.tensor_tensor(
                    ao_sb[0:64, hp, ssl], pv[0][0:64, :], lin[0:64, :],
                    mybir.AluOpType.mult,
                )
                nc.vector.tensor_tensor(
                    ao_sb[64:128, hp, ssl], pv[1][64:128, :], lin[64:128, :],
                    mybir.AluOpType.mult,
                )

            def outproj_nb(nb, dgroup, kts=(0, 1), out_col0=None, tail=False):
                """Output projection for n-block nb, d-tiles [2*dgroup, +2),
                contracting feature-tiles `kts`, writing token-cols at
                out_col0 (defaults to the block's own columns).  In the
                drain phase (tail=True) psums come from the freed score
                pool (3 deep) and half the copies go to the Activation
                engine -- neither has attention work left there, and DVE
                alone was pacing the drain."""
                nsl = slice(nb * SB, (nb + 1) * SB)
                if out_col0 is None:
                    out_col0 = nb * SB
                osl = slice(out_col0, out_col0 + SB)
                ost = expp.tile([P, 2, SB], dt.bfloat16, tag="ost", name="ost", bufs=6)
                for u in range(2):
                    dtile = 2 * dgroup + u
                    if tail and u == 0:
                        # borrow the freed score ring so the drain runs a
                        # 4-deep psum pipeline instead of 2
                        ps = scps.tile([P, SB], dt.float32, tag="sc", name="po")
                    else:
                        ps = mmps.tile([P, SB], dt.float32, tag="ps", name="po")
                    for ki, kt in enumerate(kts):
                        nc.tensor.matmul(
                            ps,
                            lhsT=wo_sb[:, kt, dtile, :],
                            rhs=ao_sb[:, kt, nsl],
                            start=(ki == 0),
                            stop=(ki == len(kts) - 1),
                        )
                    # GPSIMD can't read psum; split Act/DVE in the tail
                    if tail and u == 0:
                        nc.scalar.copy(ost[:, u, :], ps)
                    else:
                        nc.vector.tensor_copy(ost[:, u, :], ps)
                if tail:
                    # the drain is dispatch-latency-bound: one DMA per
                    # dgroup, rotated across three sequencer queues
                    eng = (nc.sync, nc.scalar, nc.gpsimd)[dgroup % 3]
                    eng.dma_start(
                        outT_r[:, 2 * dgroup : 2 * dgroup + 2, osl], ost
                    )
                else:
                    nc.sync.dma_start(
                        outT_r[:, 2 * dgroup : 2 * dgroup + 2, osl], ost
                    )

            # ---- driver ----
            from collections import deque

            filler = deque()
            budget = [0]
            N_RES = int(os.environ.get("MHA_RESERVE", "3"))
            # nb2 outproj dgroups held for the drain window
            reserve = tuple(range(N_CT // 2 - N_RES, N_CT // 2))

            def push_block_units(nb):
                kh_halves = [None]
                qh_halves = [None]
                vh_halves = [None]

                def dma_unit(src, tag, store):
                    def u():
                        store[0] = load_st(src, nb, tag)
                    return u

                key = (nb, 0)
                filler.append((dma_unit(kT, "sk", kh_halves), C_DMA, key))
                filler.append((dma_unit(qT, "sq", qh_halves), C_DMA, key))
                filler.append((dma_unit(vT, "sv", vh_halves), C_DMA, key))
                def qk_units(halves_store, w_sb, dst, bn, ft):
                    state = [None]

                    def g1():
                        state[0] = project_qk_g1(halves_store[0], w_sb, ft)

                    def g2():
                        project_qk_g2(
                            halves_store[0], w_sb, dst, bn, nb, ft, state[0]
                        )

                    return g1, g2

                for ft in range(N_FT):
                    g1, g2 = qk_units(kh_halves, wk_sb, kh_sb, "k", ft)
                    filler.append((g1, C_QK - 1, key))
                    filler.append((g2, 1, key))
                for ft in range(N_FT):
                    g1, g2 = qk_units(qh_halves, wq_sb, qh_sb, "q", ft)
                    filler.append((g1, C_QK - 1, key))
                    filler.append((g2, 1, key))
                def v_units(m):
                    state = [None]

                    def g1():
                        state[0] = project_v_g1(vh_halves[0], m)

                    def g2():
                        project_v_g2(vh_halves[0], nb, m, state[0])

                    return g1, g2

                for m in range(4):
                    g1, g2 = v_units(m)
                    filler.append((g1, C_V - 1, key))
                    filler.append((g2, 1, key))

            # block (·, 0) prerequisites run serially; everything later is
            # interleaved into the attention stream.  DMA issue order
            # interleaves weight halves with token halves so the first
            # k-proj matmul (needs wk half 0 + k-tokens half 0) starts
            # ~3us in.
            def w_rearr(w):
                if FP8:
                    return w.rearrange("(ct p) (two f) -> p ct two f", p=P, two=2)
                return w.rearrange("(ct p) f -> p ct f", p=P)

            # k-path first: wk quarter 0, k quarter 0, then the rest of wk
            # in one DMA (fewer HWDGE slots), remaining k quarters; wq/wv as
            # single DMAs; wo (1MB, first needed by outproj ~25us in) is
            # deferred into the filler so block-0 v tokens land earlier.
            wkr = w_rearr(wk)
            kh0 = []

            def load_k0_quarter(h):
                if FP8:
                    sth = stA.tile([P, 2, 2, SB], IN_DT, tag=f"sk{h}", name=f"sk{h}")
                    nc.sync.dma_start(
                        sth,
                        kT.rearrange("(ct p) n2 -> p ct n2", p=P)[
                            :, 2 * h : 2 * h + 2, 0 : 2 * SB
                        ],
                    )
                else:
                    sth = stA.tile([P, 2, SB], IN_DT, tag=f"sk{h}", name=f"sk{h}")
                    nc.sync.dma_start(
                        sth,
                        kT[:, 0:SB].rearrange("(ct p) n -> p ct n", p=P)[
                            :, 2 * h : 2 * h + 2
                        ],
                    )
                kh0.append(sth)

            nc.sync.dma_start(wk_sb[:, 0:2], wkr[:, 0:2])
            load_k0_quarter(0)
            nc.sync.dma_start(wk_sb[:, 2:8], wkr[:, 2:8])
            for h in range(1, 4):
                load_k0_quarter(h)
            nc.sync.dma_start(wq_sb, w_rearr(wq))
            qh0 = load_st(qT, 0, "sq")
            nc.sync.dma_start(wv_sb, w_rearr(wv))
            vh0 = load_st(vT, 0, "sv")
            if n_patterns:
                nc.sync.dma_start(mask_sb, maskt.rearrange("m p s -> p m s"))
            if has_bias:
                nc.sync.dma_start(
                    bias_sb["q"], bq.rearrange("(ft p) o -> p (ft o)", p=P)
                )
                nc.sync.dma_start(
                    bias_sb["k"], bk.rearrange("(ft p) o -> p (ft o)", p=P)
                )
            for ft in range(N_FT):
                project_qk(kh0, wk_sb, kh_sb, "k", 0, ft)
            for ft in range(N_FT):
                project_qk(qh0, wq_sb, qh_sb, "q", 0, ft)
            # j=0 scores need only kh/qh: emitting them here (before the
            # v-projections in the PE stream) fills the wait for the
            # block-0 v-token DMAs with useful score+exp work; the PVs in
            # attention_block consume them as phase-A records.
            PRO_HOIST = int(os.environ.get("MHA_PRO_HOIST", "0"))
            if PRO_HOIST:
                for php in range(2):
                    ilist0, trim0 = block_order(php, 0)
                    for i in ilist0[:PRO_HOIST]:
                        hoisted.setdefault((php, 0), []).append(
                            emit_score_exp(php, 0, i, trim0)
                        )
            for m in range(4):
                project_v(vh0, 0, m)
            def push_hoist_units(j):
                for hp in range(2):
                    ilist, trim = block_order(hp, j)

                    for i in ilist[:HOIST]:
                        filler.append(
                            (
                                lambda hp=hp, j=j, i=i, trim=trim: hoisted.setdefault(
                                    (hp, j), []
                                ).append(emit_score_exp(hp, j, i, trim)),
                                C_HS,
                                (j, hp),
                            )
                        )

            start_b3 = 0
            for nb in range(1, N_NB):
                if nb == N_NB - 1:
                    start_b3 = len(filler)
                push_block_units(nb)
                if nb >= int(os.environ.get("MHA_HOIST_MIN_J", "2")):
                    push_hoist_units(nb)

            def load_wo():
                nc.sync.dma_start(
                    wo_sb, wo.rearrange("(kt p) (dt q) -> p kt dt q", p=P, q=P)
                )

            # after block-1's input DMAs (k1 feeds j=1 scores), before its
            # projection units; forced by the start of j=1 at the latest
            filler.insert(start_b3 + 2, (load_wo, C_DMA, (1, 0)))
            last = N_SB - 1
            for j in range(N_SB):
                attention_block(0, j)
                if j == last:
                    # kt=0 half of the last block's outproj runs as filler
                    # during the final attention block; kt=1 follows after
                    # (host sums the two partials).  Two nb2 units are held
                    # in reserve behind them (cost 99 blocks budget-drain)
                    # to keep PE hot through the final normalize chain.
                    C_KT0 = int(os.environ.get("MHA_C_KT0", "1"))
                    for dg in range(N_CT // 2):
                        filler.append(
                            (lambda g=dg: outproj_nb(last, g, kts=(0,)), C_KT0, None)
                        )
                    for g in reserve:
                        # the key is never force-flushed and the huge cost
                        # is never budget-drained -- these pop only in the
                        # final drain loop, after the last normalize is
                        # emitted, keeping PE hot through that chain
                        filler.append(
                            (
                                lambda x=g: outproj_nb(last - 1, x, tail=True),
                                10**9,
                                (999, 0),
                            )
                        )
                attention_block(1, j)
                if j == last:
                    for dg in range(N_CT // 2):
                        filler.append(
                            (
                                lambda g=dg: outproj_nb(
                                    last, g, kts=(1,), out_col0=NTOK_LOC,
                                    tail=True,
                                ),
                                1,
                                None,
                            )
                        )
                else:
                    dgs = range(N_CT // 2)
                    if j == last - 1:
                        dgs = range(N_CT // 2 - len(reserve))
                    for dg in dgs:
                        # higher cost spreads outproj into the
                        # (filler-starved) late attention rounds
                        filler.append((lambda x=j, g=dg: outproj_nb(x, g), C_OP, None))
            while filler:
                filler.popleft()[0]()

    nc.compile()
    return nc


_NC_CACHE = {}


def _get_nc(kind_key, kind, mix_idx, n_patterns, has_bias):
    key = (kind_key, n_patterns, has_bias)
    if key not in _NC_CACHE:
        _NC_CACHE[key] = _build_nc(kind, mix_idx, n_patterns, has_bias)
    return _NC_CACHE[key]


F8 = ml_dtypes.float8_e4m3


def _split8(x):
    """fp32 [D, N] -> [D, 2N] fp8e4 with (hi, lo) interleaved per
    512-token block: layout [D, nb, {hi, lo}, SB]."""
    d = x.shape[0]
    hi = x.astype(F8)
    lo = (x - hi.astype(np.float32)).astype(F8)
    a = np.stack(
        [hi.reshape(d, -1, SB), lo.reshape(d, -1, SB)], axis=2
    )  # [D, nb, 2, SB]
    return np.ascontiguousarray(a.reshape(d, -1))


def _splitw8(wT):
    """Weight [D, F] fp32, pre-scaled: -> [D, 2F] fp8 in (lo, hi) order."""
    ws = wT * W_SCALE
    hi = ws.astype(F8)
    lo = (ws - hi.astype(np.float32)).astype(F8)
    return np.ascontiguousarray(
        np.stack([lo, hi], axis=1).reshape(wT.shape[0], -1)
    )


def kernel(v, k, q, mask, Wq, bq, Wk, bk, Wv, bv, Wo, bo, trace=False):
    global LAST_RESULTS, LAST_EXEC_WALL
    from concourse.bass_utils import run_bass_kernel_spmd

    in_np = ml_dtypes.bfloat16

    def prep_T(x):  # [S, D] -> [D, S] in input dtype (or fp8 hi/lo pair)
        xT = np.ascontiguousarray(np.asarray(x, dtype=np.float32).T)
        if FP8:
            return _split8(xT)
        return xT.astype(in_np)

    kind, mix_idx, patterns = _mask_structure(np.asarray(mask, dtype=np.float32))
    maskt = (
        np.ascontiguousarray(np.stack(patterns)).astype(ml_dtypes.bfloat16)
        if patterns
        else None
    )

    has_bias = bool(np.any(np.asarray(bq)) or np.any(np.asarray(bk)))
    kind_key = str(kind) + str(mix_idx)
    nc = _get_nc(kind_key, kind, mix_idx, len(patterns), has_bias)

    q_np = np.asarray(q, np.float32)
    k_np = np.asarray(k, np.float32)
    v_np = np.asarray(v, np.float32)
    qT = [prep_T(q_np[b]) for b in range(BATCH)]
    kT = [prep_T(k_np[b]) for b in range(BATCH)]
    vT = [prep_T(v_np[b]) for b in range(BATCH)]

    in_maps = []
    for core in range(N_CORES):
        b = core // 4
        hg = core % 4
        fsl = slice(hg * FW, (hg + 1) * FW)
        def prep_w(W):
            wT = np.ascontiguousarray(np.asarray(W, np.float32)[fsl].T)
            if FP8:
                return _splitw8(wT)
            return wT.astype(in_np)

        wo_scale = W_SCALE if FP8 else 1.0
        m = {
            "qT": qT[b],
            "kT": kT[b],
            "vT": vT[b],
            "wqT": prep_w(Wq),
            "wkT": prep_w(Wk),
            "wvT": prep_w(Wv),
            "woT": np.ascontiguousarray(
                np.asarray(Wo, np.float32)[:, fsl].T / wo_scale
            ),
        }
        if maskt is not None:
            m["maskt"] = maskt
        if has_bias:
            # projections are scaled by W_SCALE in fp8 mode; scale the
            # biases to match (the exp scale folds it back out)
            m["bq"] = np.asarray(bq, np.float32)[fsl].reshape(FW, 1) * wo_scale
            m["bk"] = np.asarray(bk, np.float32)[fsl].reshape(FW, 1) * wo_scale
        in_maps.append(m)

    import time as _time

    _t0 = _time.time()
    res = run_bass_kernel_spmd(
        nc, in_maps, core_ids=list(range(N_CORES)), trace=trace
    )
    LAST_EXEC_WALL = _time.time() - _t0
    LAST_RESULTS = res

    out = np.zeros((BATCH, SEQ, D_MODEL), dtype=np.float32)
    last0 = (N_NB - 1) * SB
    for core in range(N_CORES):
        b = core // 4
        oT = res.results[core]["outT"]
        out[b] += oT[:, :NTOK_LOC].T
        # kt=1 partial of the last n-block lives in the extra columns
        out[b, last0 : last0 + SB] += oT[:, NTOK_LOC:].T
    # v-bias contributes the constant bv @ Wo.T (softmax rows sum to 1)
    out += (
        np.asarray(bo, np.float32)
        + np.asarray(bv, np.float32) @ np.asarray(Wo, np.float32).T
    )[None, None, :]
    return out

